# revision 22
# baseline (speedup 1.0000x reference)
"""Causal self-attention Trainium2 kernel (B=8, T=2048, C=256, H=4).

Sharding: batch B=8 across the 8 NeuronCores (data parallel, no collectives).
Each core computes one batch element end-to-end:
  qkv = x @ W_attn ; per-head causal softmax(q k^T / sqrt(hs)) @ v ; @ W_proj

Layout strategy (per core):
  - x streamed in 4 token chunks of 512: per chunk DMA -> bf16 cast (DVE)
    -> PE transpose -> q/k matmuls (drained on ACT) -> v matmuls (drained
    ACT/DVE split for setup balance).
  - qT,kT computed transposed (feature rows on partitions); the softmax
    scale * log2(e) is folded into the W_attn q-columns at the bf16 cast
    so scores come out of the PE in log2 units.
  - S^T tiles (k on partitions, q on free dim) = kT_tile.T @ qT_block; the
    two heads of a pair are emitted back-to-back with K=64 row groups 0/64
    so they pack concurrently in the PE array. Diagonal tiles are TRIMMED:
    only the q >= key-block columns are computed (width w = 512-off), the
    two heads' trimmed tiles packed adjacently in sg/pg.
  - exp2 is SPLIT across two engines, greedy-balanced per group:
      * ACT: activation(Exp, scale=ln2) -> 2^s exact.
      * DVE: Schraudolph bit-trick, one tensor_scalar:
        int16(s*128 + (127-sigma)*128) bitcast as bf16 == 2^(s-sigma) with
        +-2.5% quasi-random error; sigma = 1.5-1/ln2 centers the mean
        multiplicative bias so mixing with exact-exp keys stays unbiased
        (softmax normalization cancels any common factor).
  - causal mask on diagonal 128x128 blocks via gpsimd affine_select on P
    (triangle at cols 0:128 of each trimmed head segment).
  - O^T += V_tile.T @ P (per-head accumulators, M=65: the 65th stationary
    column is ones so row 64 of O^T accumulates the softmax row sums).
  - Software pipelining: emission order scores(g), scores(g+1), PV(g);
    oacc bufs=4 so a new round's PV never waits on normalization reads.
  - normalization (per round, deferred a few groups to avoid in-order
    queue stalls): ACT extracts the two sum rows from PSUM row 64 ->
    [1,1024] SBUF; idle DMA engines broadcast them across partitions
    ([128,512], head0 rows 0-63 / head1 rows 64-127); one
    reciprocal_approx_fast [128,512]; two PSUM-direct muls into yT
    (the second with a 32-aligned partition shift 0-63 -> 64-127).
  - proj tail: z = Y @ W_proj, deep-buffered; z drains split ACT/DVE.
"""

import sys

if "/opt/trn_rl_repo" not in sys.path:
    sys.path.insert(0, "/opt/trn_rl_repo")

import numpy as np

import concourse.bass as bass
import concourse.mybir as mybir
from concourse import bacc
from concourse.masks import make_identity
from concourse.tile import TileContext

B, T, C = 8, 2048, 256
H, HS = 4, 64
NT = T // 128            # 16 token tiles
NQB = T // 512           # 4 q blocks of 512
F32 = mybir.dt.float32
BF16 = mybir.dt.bfloat16
I16 = mybir.dt.int16
LOG2E = 1.4426950408889634
LN2 = 0.6931471805599453
QSCALE = LOG2E / 8.0     # softmax scale 1/sqrt(hs) in log2 units
SIGMA = 1.5 - 1.0 / np.log(2.0)       # centers Schraudolph mean bias
EXPB = float((127.0 - SIGMA) * 128.0)  # bf16-space exp2 bias

import os
NORM_V2 = os.environ.get("KRN_NORM_V2", "1") == "1"   # new normalize path
EXP_SPLIT = os.environ.get("KRN_EXP_SPLIT", "1") == "1"  # DVE exp share
TRIM = os.environ.get("KRN_TRIM", "1") == "1"         # diagonal trimming
DRAINSPLIT = os.environ.get("KRN_DRAINSPLIT", "1") == "1"  # v/z ACT drains

_cached_nc = None


def _build():
    nc = bacc.Bacc("TRN2", target_bir_lowering=False, debug=False)
    x_d = nc.declare_dram_parameter("x", [T, C], F32, isOutput=False)
    wa_d = nc.declare_dram_parameter("W_attn", [C, 3 * C], F32, isOutput=False)
    wp_d = nc.declare_dram_parameter("W_proj", [C, C], F32, isOutput=False)
    y_d = nc.declare_dram_parameter("y", [T, C], F32, isOutput=True)

    with TileContext(nc) as tc:
        sb = tc.alloc_tile_pool(name="sb", bufs=1)
        x_c = [sb.tile([128, 1024], F32, name=f"x{c}") for c in range(4)]
        xb_c = [sb.tile([128, 1024], BF16, name=f"xb{c}") for c in range(4)]
        xT_c = [sb.tile([128, 1024], BF16, name=f"xT{c}") for c in range(4)]
        qTt = [[sb.tile([128, 512], BF16, name=f"qT{fh}_{nb}")
                for nb in range(NQB)] for fh in range(2)]
        kTt = [[sb.tile([128, 512], BF16, name=f"kT{fh}_{nb}")
                for nb in range(NQB)] for fh in range(2)]
        v65c = [sb.tile([128, 4 * 260], BF16, name=f"v65_{c}")
                for c in range(4)]
        yTt = [[sb.tile([128, 512], BF16, name=f"yT{hp}_{tqb}")
                for tqb in range(NQB)] for hp in range(2)]
        wa_f = sb.tile([128, 2 * 768], F32, name="wa_f")
        wa_b = sb.tile([128, 2 * 768], BF16, name="wa_b")
        wp_f = sb.tile([128, 2 * 256], F32, name="wp_f")
        wp_b = sb.tile([128, 2 * 256], BF16, name="wp_b")
        ident = sb.tile([128, 128], F32, name="ident")
        identb = sb.tile([128, 128], BF16, name="identb")

        make_identity(nc, ident)
        nc.vector.tensor_copy(identb, ident)
        for c in range(4):
            nc.gpsimd.memset(v65c[c], 1.0)  # ones cols survive the v copies

        # ---- load inputs: x chunk 0 first so the DVE cast chain starts
        # early; weight casts go to ACT (its free affine folds QSCALE)
        for hh in range(2):
            nc.sync.dma_start(
                x_c[0][:, hh * 512:(hh + 1) * 512].rearrange(
                    "p (n c2) -> p n c2", n=2),
                x_d[hh * 256:(hh + 1) * 256].rearrange(
                    "(n p) c2 -> p n c2", p=128),
            )
        nc.sync.dma_start(
            wa_f.rearrange("p (k m) -> p k m", k=2),
            wa_d[:].rearrange("(k p) m -> p k m", p=128),
        )
        nc.sync.dma_start(
            wp_f.rearrange("p (k m) -> p k m", k=2),
            wp_d[:].rearrange("(k p) m -> p k m", p=128),
        )
        for c in range(1, 4):
            nc.sync.dma_start(
                x_c[c].rearrange("p (n c2) -> p n c2", n=4),
                x_d[c * 512:(c + 1) * 512].rearrange(
                    "(n p) c2 -> p n c2", p=128),
            )
        nc.vector.tensor_copy(xb_c[0][:, 0:512], x_c[0][:, 0:512])
        nc.vector.tensor_copy(xb_c[0][:, 512:1024], x_c[0][:, 512:1024])
        for kc in range(2):
            nc.scalar.activation(
                wa_b[:, kc * 768: kc * 768 + 256],
                wa_f[:, kc * 768: kc * 768 + 256],
                mybir.ActivationFunctionType.Copy, scale=QSCALE,
            )
            nc.scalar.activation(
                wa_b[:, kc * 768 + 256: kc * 768 + 768],
                wa_f[:, kc * 768 + 256: kc * 768 + 768],
                mybir.ActivationFunctionType.Copy,
            )
        for c in range(1, 4):
            nc.vector.tensor_copy(xb_c[c], x_c[c])
        nc.scalar.activation(wp_b[:], wp_f[:],
                             mybir.ActivationFunctionType.Copy)

        # ---- streamed setup: per chunk cast -> transpose -> qkv ----
        with tc.tile_pool(name="pset", bufs=1, space="PSUM") as pset:
            for c in range(4):
                for kc in range(2):
                    tp = pset.tile([128, 512], BF16, tag="tp", bufs=2)
                    for j in range(4):
                        nc.tensor.transpose(
                            tp[:, j * 128:(j + 1) * 128],
                            xb_c[c][:, j * 256 + kc * 128:
                                    j * 256 + kc * 128 + 128],
                            identb,
                        )
                    nc.vector.tensor_copy(
                        xT_c[c][:, kc * 512:(kc + 1) * 512], tp[:]
                    )
                # q/k for this token block (nb == c); drains on ACT
                for fh in range(2):
                    ps_q = pset.tile([128, 512], F32, tag="mm", bufs=2)
                    nc.tensor.matmul(
                        ps_q,
                        wa_b[:, 0 * 768 + fh * 128: 0 * 768 + fh * 128 + 128],
                        xT_c[c][:, 0:512], start=True, stop=False,
                    )
                    nc.tensor.matmul(
                        ps_q,
                        wa_b[:, 1 * 768 + fh * 128: 1 * 768 + fh * 128 + 128],
                        xT_c[c][:, 512:1024], start=False, stop=True,
                    )
                    nc.scalar.activation(
                        qTt[fh][c][:], ps_q, mybir.ActivationFunctionType.Copy,
                    )
                    ps_k = pset.tile([128, 512], F32, tag="mm", bufs=2)
                    nc.tensor.matmul(
                        ps_k,
                        wa_b[:, 0 * 768 + 256 + fh * 128:
                             0 * 768 + 256 + fh * 128 + 128],
                        xT_c[c][:, 0:512], start=True, stop=False,
                    )
                    nc.tensor.matmul(
                        ps_k,
                        wa_b[:, 1 * 768 + 256 + fh * 128:
                             1 * 768 + 256 + fh * 128 + 128],
                        xT_c[c][:, 512:1024], start=False, stop=True,
                    )
                    nc.scalar.activation(
                        kTt[fh][c][:], ps_k, mybir.ActivationFunctionType.Copy,
                    )
                # v for the 4 token tiles of this chunk (drains split
                # ACT/DVE to balance the setup phase)
                for nl in range(4):
                    ps_v = pset.tile([128, 256], F32, tag="mm", bufs=2)
                    for kc in range(2):
                        nc.tensor.matmul(
                            ps_v,
                            xT_c[c][:, kc * 512 + nl * 128:
                                    kc * 512 + nl * 128 + 128],
                            wa_b[:, kc * 768 + 512: kc * 768 + 768],
                            start=(kc == 0),
                            stop=(kc == 1),
                        )
                    v_dst = v65c[c][:, nl * 260: nl * 260 + 260].rearrange(
                        "p (g c2) -> p g c2", g=4)[:, :, 0:64]
                    v_src = ps_v.rearrange("p (g c2) -> p g c2", g=4)
                    if (nl % 2 == 0) or not DRAINSPLIT:
                        nc.vector.tensor_copy(v_dst, v_src)
                    else:
                        nc.scalar.activation(
                            v_dst, v_src, mybir.ActivationFunctionType.Copy,
                        )

        # ---- attention: software-pipelined scores/exp(2 engines)/PV ----
        # greedy engine balance clocks (ns), per the engine cost models
        eng_clock = {"act": 1500.0, "dve": 0.0}

        def act_cost(w2):
            return (w2 + 172) / 1.2

        def dve_cost(w2):
            return (w2 + 120) / 0.96

        with tc.tile_pool(name="pat", bufs=1, space="PSUM") as pat:
            items = []
            round_start = []             # item index where each round begins
            for tqb in range(NQB):       # ascending: choppy small rounds
                for hp in range(2):      # sit at the start, the end is the
                    ntk = 4 * (tqb + 1)  # long smooth t3 rounds
                    round_start.append(len(items))
                    groups = list(range(ntk))   # group = both heads of one tk
                    for gi, tk in enumerate(groups):
                        off = (tk - 4 * tqb) * 128 if tk >= 4 * tqb else 0
                        items.append({
                            "hp": hp, "tqb": tqb, "tk": tk, "ntk": ntk,
                            "rnd": len(round_start) - 1,
                            "off": off, "w": (512 - off) if TRIM else 512,
                            "first": gi == 0, "last": gi == len(groups) - 1,
                        })
            round_start.append(len(items) + 1)
            round_start.append(len(items) + 1)

            def emit_scores_exp(it):
                hp, tqb, tk, off, w = (
                    it["hp"], it["tqb"], it["tk"], it["off"], it["w"])
                sg = pat.tile([128, 1024], F32, tag="sg", bufs=3)
                pg = sb.tile([128, 1024], BF16, tag="P", bufs=8, name="pg")
                woff = 512 - w   # 0 when trim disabled
                for h in range(2):
                    # head h lives at fixed col h*512 (PSUM-bank aligned);
                    # trimming shortens only the width
                    nc.tensor.matmul(
                        sg[:, h * 512: h * 512 + w],
                        kTt[hp][tk // 4][64 * h: 64 * h + 64,
                                         (tk % 4) * 128:(tk % 4) * 128 + 128],
                        qTt[hp][tqb][64 * h: 64 * h + 64, woff:512],
                        start=True, stop=True,
                    )
                w2 = 2 * w
                ca, cd = act_cost(w2), dve_cost(w2)
                pg_ap = pg.rearrange("p (g c) -> p g c", g=2)[:, :, 0:w]
                sg_ap = sg.rearrange("p (g c) -> p g c", g=2)[:, :, 0:w]
                if (not EXP_SPLIT) or (
                        eng_clock["act"] + ca <= eng_clock["dve"] + cd):
                    eng_clock["act"] += ca
                    nc.scalar.activation(
                        pg_ap, sg_ap,
                        mybir.ActivationFunctionType.Exp, scale=LN2,
                    )
                else:
                    eng_clock["dve"] += cd
                    nc.vector.tensor_scalar(
                        pg_ap.bitcast(I16), sg_ap, 128.0, EXPB,
                        mybir.AluOpType.mult, mybir.AluOpType.add,
                    )
                if tk >= 4 * tqb:  # diagonal: zero the triangle in both
                    # heads' trimmed tiles with one select (outer stride-0
                    # pattern repeats the same triangle w cols apart)
                    tri0 = 0 if TRIM else off
                    dv = pg.rearrange("p (g c) -> p g c", g=2,
                                      )[:, :, tri0:tri0 + 128]
                    nc.gpsimd.affine_select(
                        out=dv, in_=dv,
                        compare_op=mybir.AluOpType.is_ge,
                        fill=0.0,
                        base=0,
                        pattern=[[0, 2], [1, 128]],
                        channel_multiplier=-1,
                    )
                it["pg"] = pg

            def emit_pv(it, acc):
                hp, tqb, tk, ntk = it["hp"], it["tqb"], it["tk"], it["ntk"]
                off, w, pg = it["off"], it["w"], it["pg"]
                for h in range(2):
                    gh = 2 * hp + h
                    pvo = 0 if TRIM else off
                    nc.tensor.matmul(
                        acc[h][0:65, off:],
                        v65c[tk // 4][:, (tk % 4) * 260 + gh * 65:
                                      (tk % 4) * 260 + gh * 65 + 65],
                        pg[:, h * 512 + pvo: h * 512 + w],
                        start=(tk == 0), stop=(tk == ntk - 1),
                    )

            def emit_norm_head(acc, hp, tqb):
                """Stage 1 (right after last PV): ACT stages O^T rows + sums
                to SBUF bf16 -- one hop frees the oacc buffers; DMA
                broadcasts the sum rows across partitions."""
                oc = sb.tile([128, 1024], F32, tag="ocopy", bufs=3,
                             name="oc")
                for h in range(2):
                    nc.scalar.activation(
                        oc[0:65, h * 512:(h + 1) * 512], acc[h][0:65, :],
                        mybir.ActivationFunctionType.Copy,
                    )
                eng_clock["act"] += 2 * act_cost(512)
                return oc

            def emit_proj(tqb):
                """Projection for one q block, interleaved into the
                attention stream; psz borrows one sg-tag PSUM slot so it
                never WAW-serializes against the O^T accumulators."""
                psz = pat.tile([128, 1024], F32, tag="sg", bufs=3,
                               name="psz")
                for half in range(2):
                    for sub in range(2):
                        nloc = half * 2 + sub
                        for fh in range(2):
                            nc.tensor.matmul(
                                psz[:, half * 512 + sub * 256:
                                    half * 512 + (sub + 1) * 256],
                                yTt[fh][tqb][:, nloc * 128:(nloc + 1) * 128],
                                wp_b[:, fh * 256: fh * 256 + 256],
                                start=(fh == 0),
                                stop=(fh == 1),
                            )
                for half in range(2):
                    z_sb = sb.tile([128, 512], F32, tag="z", bufs=8,
                                   name="z_sb")
                    if half == 0 or not DRAINSPLIT:
                        nc.vector.tensor_copy(
                            z_sb, psz[:, half * 512:(half + 1) * 512])
                        eng_clock["dve"] += (512 + 120) / 0.96
                    else:
                        nc.scalar.activation(
                            z_sb, psz[:, half * 512:(half + 1) * 512],
                            mybir.ActivationFunctionType.Copy,
                        )
                        eng_clock["act"] += act_cost(512)
                    n0 = tqb * 4 + half * 2
                    nc.sync.dma_start(
                        y_d[:].rearrange(
                            "(n p) c -> p n c", p=128)[:, n0: n0 + 2],
                        z_sb.rearrange("p (n c) -> p n c", n=2),
                    )

            def emit_norm_tail(acc, oc, hp, tqb):
                """Stage 2 (a full round later, PSUM-free): reciprocal of the
                sums row, gpsimd broadcast of the reciprocals, two muls."""
                yt = yTt[hp][tqb]
                srow = sb.tile([1, 1024], F32, tag="srow", bufs=3,
                               name="srow")
                rrow = sb.tile([1, 1024], F32, tag="rrow", bufs=3, name="rrow")
                rb = sb.tile([64, 1024], F32, tag="recip", bufs=3, name="rb")
                nc.vector.tensor_copy(srow[0:1, :], oc[64:65, :])
                nc.vector.reciprocal_approx_fast(rrow[0:1, :], srow[0:1, :])
                for h in range(2):
                    nc.gpsimd.partition_broadcast(
                        rb[0:64, h * 512:(h + 1) * 512],
                        rrow[0:1, h * 512:(h + 1) * 512], channels=64)
                nc.vector.tensor_mul(yt[0:64, :], oc[0:64, 0:512],
                                     rb[0:64, 0:512])
                nc.vector.tensor_mul(yt[64:128, :], oc[0:64, 512:1024],
                                     rb[0:64, 512:1024])
                eng_clock["dve"] += dve_cost(1024) + 2 * ((512 + 151) / 0.96)

            def emit_normalize_v1(acc, hp, tqb):
                """Baseline normalize: stage O^T + sums to SBUF, gpsimd
                broadcast, recip, two muls."""
                yt = yTt[hp][tqb]
                oc = sb.tile([128, 1024], F32, tag="ocopy", bufs=2, name="oc")
                nc.vector.tensor_copy(oc[0:65, 0:512], acc[0][0:65, :])
                nc.vector.tensor_copy(oc[0:65, 512:1024], acc[1][0:65, :])
                srow = sb.tile([1, 1024], F32, tag="srow", bufs=2, name="srow")
                nc.vector.tensor_copy(srow[0:1, :], oc[64:65, :])
                sr = sb.tile([128, 1024], F32, tag="bcast1", bufs=2, name="sr")
                nc.gpsimd.partition_broadcast(sr[0:64, :], srow[0:1, :],
                                              channels=64)
                rb = sb.tile([128, 1024], F32, tag="recip1", bufs=2, name="rb")
                nc.vector.reciprocal_approx_fast(rb[0:64, :], sr[0:64, :])
                nc.vector.tensor_mul(yt[0:64, :], oc[0:64, 0:512],
                                     rb[0:64, 0:512])
                nc.vector.tensor_mul(
                    yt[64:128, :], oc[0:64, 512:1024], rb[0:64, 512:1024]
                )

            PV_LAG = 2     # scores/exp run this many groups ahead of PV
            acc = None
            pending = []   # deferred normalize stages: (due_idx, fn)
            for idx in range(len(items) + PV_LAG):
                if idx < len(items):
                    emit_scores_exp(items[idx])
                j = idx - PV_LAG
                if j >= 0:
                    it = items[j]
                    if it["first"]:
                        oa = pat.tile([128, 512], F32, tag="oacc", bufs=2,
                                      name="oa")
                        ob = pat.tile([128, 512], F32, tag="oacc", bufs=2,
                                      name="ob")
                        acc = (oa, ob)
                    emit_pv(it, acc)
                    if it["last"]:
                        a_, hp_, tqb_ = acc, it["hp"], it["tqb"]
                        if not NORM_V2:
                            emit_normalize_v1(a_, hp_, tqb_)
                        else:
                            def mk_head(a=a_, hp=hp_, tqb=tqb_):
                                st = {}

                                def head():
                                    st["oc"] = emit_norm_head(a, hp, tqb)

                                def tail():
                                    emit_norm_tail(a, st["oc"], hp, tqb)
                                    if hp == 1:
                                        emit_proj(tqb)
                                return head, tail

                            head_fn, tail_fn = mk_head()
                            pending.append((idx + 1, head_fn))
                            # tail ~7 groups later: oc/srow/recip deps have
                            # comfortably drained by then on the fast engines
                            pending.append((idx + 7, tail_fn))
                # flush AFTER pv so a round's oc lands after its last PV
                while pending and pending[0][0] <= idx:
                    pending.pop(0)[1]()
            while pending:
                pending.pop(0)[1]()

        sb.release()
    nc.compile()
    return nc


def _get_nc():
    global _cached_nc
    if _cached_nc is None:
        _cached_nc = _build()
    return _cached_nc


def kernel(**inputs):
    from concourse.bass_utils import run_bass_kernel_spmd

    x = np.ascontiguousarray(np.asarray(inputs["x"], dtype=np.float32))
    wa = np.ascontiguousarray(np.asarray(inputs["W_attn"], dtype=np.float32))
    wp = np.ascontiguousarray(np.asarray(inputs["W_proj"], dtype=np.float32))
    nc = _get_nc()
    in_maps = [
        {"x": np.ascontiguousarray(x[b]), "W_attn": wa, "W_proj": wp}
        for b in range(B)
    ]
    res = run_bass_kernel_spmd(nc, in_maps, core_ids=list(range(B)))
    return np.stack([res.results[b]["y"] for b in range(B)], axis=0)


# revision 23
# speedup vs baseline: 1.0656x; 1.0656x over previous
"""Causal self-attention Trainium2 kernel (B=8, T=2048, C=256, H=4).

Sharding: batch B=8 across the 8 NeuronCores (data parallel, no collectives).
Each core computes one batch element end-to-end:
  qkv = x @ W_attn ; per-head causal softmax(q k^T / sqrt(hs)) @ v ; @ W_proj

Layout strategy (per core):
  - x streamed in 4 token chunks of 512: per chunk DMA -> bf16 cast (DVE)
    -> PE transpose -> q/k matmuls (drained on ACT) -> v matmuls (drained
    ACT/DVE split for setup balance).
  - qT,kT computed transposed (feature rows on partitions); the softmax
    scale * log2(e) is folded into the W_attn q-columns at the bf16 cast
    so scores come out of the PE in log2 units.
  - S^T tiles (k on partitions, q on free dim) = kT_tile.T @ qT_block; the
    two heads of a pair are emitted back-to-back with K=64 row groups 0/64
    so they pack concurrently in the PE array. Diagonal tiles are TRIMMED:
    only the q >= key-block columns are computed (width w = 512-off), the
    two heads' trimmed tiles packed adjacently in sg/pg.
  - exp2 is SPLIT across two engines, greedy-balanced per group:
      * ACT: activation(Exp, scale=ln2) -> 2^s exact.
      * DVE: Schraudolph bit-trick, one tensor_scalar:
        int16(s*128 + (127-sigma)*128) bitcast as bf16 == 2^(s-sigma) with
        +-2.5% quasi-random error; sigma = 1.5-1/ln2 centers the mean
        multiplicative bias so mixing with exact-exp keys stays unbiased
        (softmax normalization cancels any common factor).
  - causal mask on diagonal 128x128 blocks via gpsimd affine_select on P
    (triangle at cols 0:128 of each trimmed head segment).
  - O^T += V_tile.T @ P (per-head accumulators, M=65: the 65th stationary
    column is ones so row 64 of O^T accumulates the softmax row sums).
  - Software pipelining: emission order scores(g), scores(g+1), PV(g);
    oacc bufs=4 so a new round's PV never waits on normalization reads.
  - normalization (per round, deferred a few groups to avoid in-order
    queue stalls): ACT extracts the two sum rows from PSUM row 64 ->
    [1,1024] SBUF; idle DMA engines broadcast them across partitions
    ([128,512], head0 rows 0-63 / head1 rows 64-127); one
    reciprocal_approx_fast [128,512]; two PSUM-direct muls into yT
    (the second with a 32-aligned partition shift 0-63 -> 64-127).
  - proj tail: z = Y @ W_proj, deep-buffered; z drains split ACT/DVE.
"""

import sys

if "/opt/trn_rl_repo" not in sys.path:
    sys.path.insert(0, "/opt/trn_rl_repo")

import numpy as np

import concourse.bass as bass
import concourse.mybir as mybir
from concourse import bacc
from concourse.masks import make_identity
from concourse.tile import TileContext

B, T, C = 8, 2048, 256
H, HS = 4, 64
NT = T // 128            # 16 token tiles
NQB = T // 512           # 4 q blocks of 512
F32 = mybir.dt.float32
BF16 = mybir.dt.bfloat16
I16 = mybir.dt.int16
LOG2E = 1.4426950408889634
LN2 = 0.6931471805599453
QSCALE = LOG2E / 8.0     # softmax scale 1/sqrt(hs) in log2 units
SIGMA = 1.5 - 1.0 / np.log(2.0)       # centers Schraudolph mean bias
EXPB = float((127.0 - SIGMA) * 128.0)  # bf16-space exp2 bias

import os
NORM_V2 = os.environ.get("KRN_NORM_V2", "1") == "1"   # new normalize path
EXP_SPLIT = os.environ.get("KRN_EXP_SPLIT", "1") == "1"  # DVE exp share
TRIM = os.environ.get("KRN_TRIM", "1") == "1"         # diagonal trimming
DRAINSPLIT = os.environ.get("KRN_DRAINSPLIT", "1") == "1"  # v/z ACT drains

_cached_nc = None


def _build():
    nc = bacc.Bacc("TRN2", target_bir_lowering=False, debug=False)
    x_d = nc.declare_dram_parameter("x", [T, C], F32, isOutput=False)
    wa_d = nc.declare_dram_parameter("W_attn", [C, 3 * C], F32, isOutput=False)
    wp_d = nc.declare_dram_parameter("W_proj", [C, C], F32, isOutput=False)
    y_d = nc.declare_dram_parameter("y", [T, C], F32, isOutput=True)

    with TileContext(nc) as tc:
        sb = tc.alloc_tile_pool(name="sb", bufs=1)
        x_c = [sb.tile([128, 1024], F32, name=f"x{c}") for c in range(4)]
        xb_c = [sb.tile([128, 1024], BF16, name=f"xb{c}") for c in range(4)]
        xT_c = [sb.tile([128, 1024], BF16, name=f"xT{c}") for c in range(4)]
        qTt = [[sb.tile([128, 512], BF16, name=f"qT{fh}_{nb}")
                for nb in range(NQB)] for fh in range(2)]
        kTt = [[sb.tile([128, 512], BF16, name=f"kT{fh}_{nb}")
                for nb in range(NQB)] for fh in range(2)]
        v65c = [sb.tile([128, 4 * 260], BF16, name=f"v65_{c}")
                for c in range(4)]
        yTt = [[sb.tile([128, 512], BF16, name=f"yT{hp}_{tqb}")
                for tqb in range(NQB)] for hp in range(2)]
        wa_f = sb.tile([128, 2 * 768], F32, name="wa_f")
        wa_b = sb.tile([128, 2 * 768], BF16, name="wa_b")
        wp_f = sb.tile([128, 2 * 256], F32, name="wp_f")
        wp_b = sb.tile([128, 2 * 256], BF16, name="wp_b")
        ident = sb.tile([128, 128], F32, name="ident")
        identb = sb.tile([128, 128], BF16, name="identb")

        make_identity(nc, ident)
        nc.vector.tensor_copy(identb, ident)
        for c in range(4):
            nc.gpsimd.memset(v65c[c], 1.0)  # ones cols survive the v copies

        # ---- load inputs: x chunk 0 first so the DVE cast chain starts
        # early; weight casts go to ACT (its free affine folds QSCALE)
        for hh in range(2):
            nc.sync.dma_start(
                x_c[0][:, hh * 512:(hh + 1) * 512].rearrange(
                    "p (n c2) -> p n c2", n=2),
                x_d[hh * 256:(hh + 1) * 256].rearrange(
                    "(n p) c2 -> p n c2", p=128),
            )
        nc.sync.dma_start(
            wa_f.rearrange("p (k m) -> p k m", k=2),
            wa_d[:].rearrange("(k p) m -> p k m", p=128),
        )
        nc.sync.dma_start(
            wp_f.rearrange("p (k m) -> p k m", k=2),
            wp_d[:].rearrange("(k p) m -> p k m", p=128),
        )
        for c in range(1, 4):
            nc.sync.dma_start(
                x_c[c].rearrange("p (n c2) -> p n c2", n=4),
                x_d[c * 512:(c + 1) * 512].rearrange(
                    "(n p) c2 -> p n c2", p=128),
            )
        nc.vector.tensor_copy(xb_c[0][:, 0:512], x_c[0][:, 0:512])
        nc.vector.tensor_copy(xb_c[0][:, 512:1024], x_c[0][:, 512:1024])
        for kc in range(2):
            nc.scalar.activation(
                wa_b[:, kc * 768: kc * 768 + 256],
                wa_f[:, kc * 768: kc * 768 + 256],
                mybir.ActivationFunctionType.Copy, scale=QSCALE,
            )
            nc.scalar.activation(
                wa_b[:, kc * 768 + 256: kc * 768 + 768],
                wa_f[:, kc * 768 + 256: kc * 768 + 768],
                mybir.ActivationFunctionType.Copy,
            )
        for c in range(1, 4):
            nc.vector.tensor_copy(xb_c[c], x_c[c])
        nc.scalar.activation(wp_b[:], wp_f[:],
                             mybir.ActivationFunctionType.Copy)

        # ---- streamed setup: per chunk cast -> transpose -> qkv ----
        with tc.tile_pool(name="pset", bufs=1, space="PSUM") as pset:
            for c in range(4):
                for kc in range(2):
                    tp = pset.tile([128, 512], BF16, tag="tp", bufs=2)
                    for j in range(4):
                        nc.tensor.transpose(
                            tp[:, j * 128:(j + 1) * 128],
                            xb_c[c][:, j * 256 + kc * 128:
                                    j * 256 + kc * 128 + 128],
                            identb,
                        )
                    nc.vector.tensor_copy(
                        xT_c[c][:, kc * 512:(kc + 1) * 512], tp[:]
                    )
                # q/k for this token block (nb == c); drains on ACT
                for fh in range(2):
                    ps_q = pset.tile([128, 512], F32, tag="mm", bufs=2)
                    nc.tensor.matmul(
                        ps_q,
                        wa_b[:, 0 * 768 + fh * 128: 0 * 768 + fh * 128 + 128],
                        xT_c[c][:, 0:512], start=True, stop=False,
                    )
                    nc.tensor.matmul(
                        ps_q,
                        wa_b[:, 1 * 768 + fh * 128: 1 * 768 + fh * 128 + 128],
                        xT_c[c][:, 512:1024], start=False, stop=True,
                    )
                    nc.scalar.activation(
                        qTt[fh][c][:], ps_q, mybir.ActivationFunctionType.Copy,
                    )
                    ps_k = pset.tile([128, 512], F32, tag="mm", bufs=2)
                    nc.tensor.matmul(
                        ps_k,
                        wa_b[:, 0 * 768 + 256 + fh * 128:
                             0 * 768 + 256 + fh * 128 + 128],
                        xT_c[c][:, 0:512], start=True, stop=False,
                    )
                    nc.tensor.matmul(
                        ps_k,
                        wa_b[:, 1 * 768 + 256 + fh * 128:
                             1 * 768 + 256 + fh * 128 + 128],
                        xT_c[c][:, 512:1024], start=False, stop=True,
                    )
                    nc.scalar.activation(
                        kTt[fh][c][:], ps_k, mybir.ActivationFunctionType.Copy,
                    )
                # v for the 4 token tiles of this chunk (drains split
                # ACT/DVE to balance the setup phase)
                for nl in range(4):
                    ps_v = pset.tile([128, 256], F32, tag="mm", bufs=2)
                    for kc in range(2):
                        nc.tensor.matmul(
                            ps_v,
                            xT_c[c][:, kc * 512 + nl * 128:
                                    kc * 512 + nl * 128 + 128],
                            wa_b[:, kc * 768 + 512: kc * 768 + 768],
                            start=(kc == 0),
                            stop=(kc == 1),
                        )
                    v_dst = v65c[c][:, nl * 260: nl * 260 + 260].rearrange(
                        "p (g c2) -> p g c2", g=4)[:, :, 0:64]
                    v_src = ps_v.rearrange("p (g c2) -> p g c2", g=4)
                    if (nl % 2 == 0) or not DRAINSPLIT:
                        nc.vector.tensor_copy(v_dst, v_src)
                    else:
                        nc.scalar.activation(
                            v_dst, v_src, mybir.ActivationFunctionType.Copy,
                        )

        # ---- attention: software-pipelined scores/exp(2 engines)/PV ----
        # greedy engine balance clocks (ns), per the engine cost models
        eng_clock = {"act": 1500.0, "dve": 0.0}

        def act_cost(w2):
            return (w2 + 172) / 1.2

        def dve_cost(w2):
            return (w2 + 120) / 0.96

        with tc.tile_pool(name="pat", bufs=1, space="PSUM") as pat:
            items = []
            round_start = []             # item index where each round begins
            for tqb in range(NQB):       # ascending: choppy small rounds
                for hp in range(2):      # sit at the start, the end is the
                    ntk = 4 * (tqb + 1)  # long smooth t3 rounds
                    round_start.append(len(items))
                    groups = list(range(ntk))   # group = both heads of one tk
                    for gi, tk in enumerate(groups):
                        off = (tk - 4 * tqb) * 128 if tk >= 4 * tqb else 0
                        items.append({
                            "hp": hp, "tqb": tqb, "tk": tk, "ntk": ntk,
                            "rnd": len(round_start) - 1,
                            "off": off, "w": (512 - off) if TRIM else 512,
                            "first": gi == 0, "last": gi == len(groups) - 1,
                        })
            round_start.append(len(items) + 1)
            round_start.append(len(items) + 1)

            def emit_scores_exp(it):
                hp, tqb, tk, off, w = (
                    it["hp"], it["tqb"], it["tk"], it["off"], it["w"])
                sg = pat.tile([128, 1024], F32, tag="sg", bufs=3)
                pg = sb.tile([128, 1024], BF16, tag="P", bufs=8, name="pg")
                woff = 512 - w   # 0 when trim disabled
                for h in range(2):
                    # head h lives at fixed col h*512 (PSUM-bank aligned);
                    # trimming shortens only the width
                    nc.tensor.matmul(
                        sg[:, h * 512: h * 512 + w],
                        kTt[hp][tk // 4][64 * h: 64 * h + 64,
                                         (tk % 4) * 128:(tk % 4) * 128 + 128],
                        qTt[hp][tqb][64 * h: 64 * h + 64, woff:512],
                        start=True, stop=True,
                    )
                w2 = 2 * w
                ca, cd = act_cost(w2), dve_cost(w2)
                pg_ap = pg.rearrange("p (g c) -> p g c", g=2)[:, :, 0:w]
                sg_ap = sg.rearrange("p (g c) -> p g c", g=2)[:, :, 0:w]
                if (not EXP_SPLIT) or (
                        eng_clock["act"] + ca <= eng_clock["dve"] + cd):
                    eng_clock["act"] += ca
                    nc.scalar.activation(
                        pg_ap, sg_ap,
                        mybir.ActivationFunctionType.Exp, scale=LN2,
                    )
                else:
                    eng_clock["dve"] += cd
                    nc.vector.tensor_scalar(
                        pg_ap.bitcast(I16), sg_ap, 128.0, EXPB,
                        mybir.AluOpType.mult, mybir.AluOpType.add,
                    )
                if tk >= 4 * tqb:  # diagonal: zero the triangle in both
                    # heads' trimmed tiles with one select (outer stride-0
                    # pattern repeats the same triangle w cols apart)
                    tri0 = 0 if TRIM else off
                    dv = pg.rearrange("p (g c) -> p g c", g=2,
                                      )[:, :, tri0:tri0 + 128]
                    nc.gpsimd.affine_select(
                        out=dv, in_=dv,
                        compare_op=mybir.AluOpType.is_ge,
                        fill=0.0,
                        base=0,
                        pattern=[[0, 2], [1, 128]],
                        channel_multiplier=-1,
                    )
                it["pg"] = pg

            def emit_pv(it, acc):
                hp, tqb, tk, ntk = it["hp"], it["tqb"], it["tk"], it["ntk"]
                off, w, pg = it["off"], it["w"], it["pg"]
                for h in range(2):
                    gh = 2 * hp + h
                    pvo = 0 if TRIM else off
                    nc.tensor.matmul(
                        acc[h][0:65, off:],
                        v65c[tk // 4][:, (tk % 4) * 260 + gh * 65:
                                      (tk % 4) * 260 + gh * 65 + 65],
                        pg[:, h * 512 + pvo: h * 512 + w],
                        start=(tk == 0), stop=(tk == ntk - 1),
                    )

            def emit_norm_head(acc, hp, tqb):
                """Stage 1 (right after last PV): ACT stages O^T rows + sums
                to SBUF bf16 -- one hop frees the oacc buffers; DMA
                broadcasts the sum rows across partitions."""
                oc = sb.tile([128, 1024], F32, tag="ocopy", bufs=3,
                             name="oc")
                for h in range(2):
                    nc.scalar.activation(
                        oc[0:65, h * 512:(h + 1) * 512], acc[h][0:65, :],
                        mybir.ActivationFunctionType.Copy,
                    )
                eng_clock["act"] += 2 * act_cost(512)
                return oc

            def emit_norm_tail(acc, oc, hp, tqb):
                """Stage 2 (a full round later, PSUM-free): reciprocal of the
                sums row, gpsimd broadcast of the reciprocals, two muls."""
                yt = yTt[hp][tqb]
                srow = sb.tile([1, 1024], F32, tag="srow", bufs=3,
                               name="srow")
                rrow = sb.tile([1, 1024], F32, tag="rrow", bufs=3, name="rrow")
                rb = sb.tile([64, 1024], F32, tag="recip", bufs=3, name="rb")
                nc.vector.tensor_copy(srow[0:1, :], oc[64:65, :])
                nc.vector.reciprocal_approx_fast(rrow[0:1, :], srow[0:1, :])
                for h in range(2):
                    nc.gpsimd.partition_broadcast(
                        rb[0:64, h * 512:(h + 1) * 512],
                        rrow[0:1, h * 512:(h + 1) * 512], channels=64)
                nc.vector.tensor_mul(yt[0:64, :], oc[0:64, 0:512],
                                     rb[0:64, 0:512])
                nc.vector.tensor_mul(yt[64:128, :], oc[0:64, 512:1024],
                                     rb[0:64, 512:1024])
                eng_clock["dve"] += dve_cost(1024) + 2 * ((512 + 151) / 0.96)

            def emit_normalize_v1(acc, hp, tqb):
                """Baseline normalize: stage O^T + sums to SBUF, gpsimd
                broadcast, recip, two muls."""
                yt = yTt[hp][tqb]
                oc = sb.tile([128, 1024], F32, tag="ocopy", bufs=2, name="oc")
                nc.vector.tensor_copy(oc[0:65, 0:512], acc[0][0:65, :])
                nc.vector.tensor_copy(oc[0:65, 512:1024], acc[1][0:65, :])
                srow = sb.tile([1, 1024], F32, tag="srow", bufs=2, name="srow")
                nc.vector.tensor_copy(srow[0:1, :], oc[64:65, :])
                sr = sb.tile([128, 1024], F32, tag="bcast1", bufs=2, name="sr")
                nc.gpsimd.partition_broadcast(sr[0:64, :], srow[0:1, :],
                                              channels=64)
                rb = sb.tile([128, 1024], F32, tag="recip1", bufs=2, name="rb")
                nc.vector.reciprocal_approx_fast(rb[0:64, :], sr[0:64, :])
                nc.vector.tensor_mul(yt[0:64, :], oc[0:64, 0:512],
                                     rb[0:64, 0:512])
                nc.vector.tensor_mul(
                    yt[64:128, :], oc[0:64, 512:1024], rb[0:64, 512:1024]
                )

            PV_LAG = 2     # scores/exp run this many groups ahead of PV
            acc = None
            pending = []   # deferred normalize stages: (due_idx, fn)
            for idx in range(len(items) + PV_LAG):
                if idx < len(items):
                    emit_scores_exp(items[idx])
                j = idx - PV_LAG
                if j >= 0:
                    it = items[j]
                    if it["first"]:
                        oa = pat.tile([128, 512], F32, tag="oacc", bufs=2,
                                      name="oa")
                        ob = pat.tile([128, 512], F32, tag="oacc", bufs=2,
                                      name="ob")
                        acc = (oa, ob)
                    emit_pv(it, acc)
                    if it["last"]:
                        a_, hp_, tqb_ = acc, it["hp"], it["tqb"]
                        if not NORM_V2:
                            emit_normalize_v1(a_, hp_, tqb_)
                        else:
                            def mk_head(a=a_, hp=hp_, tqb=tqb_):
                                st = {}

                                def head():
                                    st["oc"] = emit_norm_head(a, hp, tqb)

                                def tail():
                                    emit_norm_tail(a, st["oc"], hp, tqb)
                                return head, tail

                            head_fn, tail_fn = mk_head()
                            pending.append((idx + 1, head_fn))
                            # tail ~7 groups later: oc/srow/recip deps have
                            # comfortably drained by then on the fast engines
                            pending.append((idx + 7, tail_fn))
                # flush AFTER pv so a round's oc lands after its last PV
                while pending and pending[0][0] <= idx:
                    pending.pop(0)[1]()
            while pending:
                pending.pop(0)[1]()

        # ---- output projection (deep-buffered tail) ----
        with tc.tile_pool(name="ppr", bufs=1, space="PSUM") as ppr:
            for tqb in range(NQB):
                for half in range(2):
                    psz = ppr.tile([128, 512], F32, tag="pz", bufs=4)
                    for sub in range(2):
                        nloc = half * 2 + sub
                        for fh in range(2):
                            nc.tensor.matmul(
                                psz[:, sub * 256:(sub + 1) * 256],
                                yTt[fh][tqb][:, nloc * 128:(nloc + 1) * 128],
                                wp_b[:, fh * 256: fh * 256 + 256],
                                start=(fh == 0),
                                stop=(fh == 1),
                            )
                    z_sb = sb.tile([128, 512], F32, tag="z", bufs=8,
                                   name="z_sb")
                    if half == 0 or not DRAINSPLIT:
                        nc.vector.tensor_copy(z_sb, psz)
                    else:
                        nc.scalar.activation(
                            z_sb, psz, mybir.ActivationFunctionType.Copy,
                        )
                    n0 = tqb * 4 + half * 2
                    nc.sync.dma_start(
                        y_d[:].rearrange(
                            "(n p) c -> p n c", p=128)[:, n0: n0 + 2],
                        z_sb.rearrange("p (n c) -> p n c", n=2),
                    )
        sb.release()
    nc.compile()
    return nc


def _get_nc():
    global _cached_nc
    if _cached_nc is None:
        _cached_nc = _build()
    return _cached_nc


def kernel(**inputs):
    from concourse.bass_utils import run_bass_kernel_spmd

    x = np.ascontiguousarray(np.asarray(inputs["x"], dtype=np.float32))
    wa = np.ascontiguousarray(np.asarray(inputs["W_attn"], dtype=np.float32))
    wp = np.ascontiguousarray(np.asarray(inputs["W_proj"], dtype=np.float32))
    nc = _get_nc()
    in_maps = [
        {"x": np.ascontiguousarray(x[b]), "W_attn": wa, "W_proj": wp}
        for b in range(B)
    ]
    res = run_bass_kernel_spmd(nc, in_maps, core_ids=list(range(B)))
    return np.stack([res.results[b]["y"] for b in range(B)], axis=0)
